# revision 19
# baseline (speedup 1.0000x reference)
"""Trainium2 Bass kernel for nn_LstmCellS (matrix-state LSTM cell).

Math (per gate g in [f, i, o, c]):
    pre[g] = hidden_u @ Ww[g]^T - x @ Wd[g]^T + hidden_s @ Wu[g]^T + (Bw+Bd+Bu)[g]
    f, i, o = sigmoid(pre[0..2]);  gg = tanh(pre[3])
    c     = f * hidden_c + i * gg
    out_s = o * tanh(c)

Sharding: tensor-parallel over the output axis p (flattened (a, b), S^2 = 4096
-> 512 per core).  Every core sees the full batch and full contraction but only
its 512-wide output slice of every gate, so the whole LSTM epilogue is local —
no collectives.  Host concatenates the 8 output slices.

Mixed precision: the f/i/o gates go through sigmoid (derivative <= 0.25), so
fp8-e4m3 quantization noise in their pre-activations is strongly attenuated;
the c-candidate gate goes through tanh (derivative ~1) and dominates the error.
Hence f/i/o matmuls run in fp8 DoubleRow mode (2 contraction rows per PE cell
per cycle) and the candidate gate runs in bf16.  Host-simulated rel err 1.34e-2
vs the 2e-2 budget.  fp8 operands are pre-scaled on host (activations x16,
weights x4096 — the weights are uniform in +-1/64, right at e4m3's subnormal
boundary) and the 2^-16 is folded into the epilogue activation's scale.

Gate-major stream order f, i, candidate, o: c = f*hc + i*g is computed on the
scalar/vector engines while the o-gate matmuls still run, leaving only
sigmoid(o) * tanh(c) + the output DMA in the tail.

All operands are SBUF-resident (~150 KiB of the 208 KiB per partition): weights
13.1 MiB + activations 3.9 MiB stream in ordered by first use, with no tile
recycling hazards.
"""

import sys

for _p in ("/root/.axon_site/_ro/trn_rl_repo", "/opt/trn_rl_repo"):
    if _p not in sys.path:
        sys.path.append(_p)

import ml_dtypes
import numpy as np

B = 256          # batch
S2 = 4096        # S*S (flattened matrix state)
U = 512          # hidden_u size
I = 512          # input size
QC = S2 + U + I  # contraction length (5120)
QT = QC // 128   # contraction tiles (40)
KP = QT // 2     # fp8 DoubleRow k-pairs (20)
QB = 32          # candidate-gate bf16 k-tiles (hidden_s section)
KG = (QT - QB) // 2  # candidate-gate fp8 k-pairs (x/hu section, 4)
NT = B // 128    # batch tiles (2)
NCORES = 8
PSH = S2 // NCORES  # output slice per core (512)

SA = 16.0        # fp8 activation scale
SW = 4096.0      # fp8 weight scale
SINV = 1.0 / (SA * SW)

F8 = ml_dtypes.float8_e4m3  # TRN float8e4: bias 7, max normal +-240
BF = ml_dtypes.bfloat16

_cache: dict = {}


def _build():
    """Build and compile the per-core Bass module (same NEFF on all cores)."""
    import concourse.tile as tile
    import concourse.mybir as mybir
    from concourse import bacc

    f32 = mybir.dt.float32
    bf16 = mybir.dt.bfloat16
    fp8 = mybir.dt.float8e4
    AF = mybir.ActivationFunctionType
    DR = mybir.MatmulPerfMode.DoubleRow

    nc = bacc.Bacc("TRN2", target_bir_lowering=False, debug=False,
                   enable_asserts=False, num_devices=NCORES)

    A8_d = nc.dram_tensor("A8", [128, QT * B], fp8, kind="ExternalInput")
    D8_d = nc.dram_tensor("D8", [128, QB * B], fp8, kind="ExternalInput")
    W8_d = nc.dram_tensor("W8", [3, 128, KP * 2 * PSH], fp8, kind="ExternalInput")
    WG_d = nc.dram_tensor("WG8", [128, KG * 2 * PSH], fp8, kind="ExternalInput")
    WB_d = nc.dram_tensor("WB", [128, QB * PSH], bf16, kind="ExternalInput")
    B_d = nc.dram_tensor("BIAS", [1, 4 * PSH], bf16, kind="ExternalInput")
    H_d = nc.dram_tensor("HC", [128, NT * PSH], bf16, kind="ExternalInput")
    OS_d = nc.dram_tensor("OS", [NT, 128, PSH], bf16, kind="ExternalOutput")
    CO_d = nc.dram_tensor("CO", [NT, 128, PSH], bf16, kind="ExternalOutput")

    with tile.TileContext(nc) as tc:
        with (
            tc.tile_pool(name="apool", bufs=1) as apool,
            tc.tile_pool(name="wpool", bufs=1) as wpool,
            tc.tile_pool(name="cpool", bufs=1) as cpool,
            tc.tile_pool(name="epool", bufs=2) as epool,
            tc.tile_pool(name="pspool", bufs=1, space="PSUM") as pspool,
        ):
            # PSUM accumulators: bank per (gate, batch-tile); gate ids f0 i1 o2 g3
            psum = [
                pspool.tile([128, PSH], f32, tag=f"ps{g}_{n}", name=f"ps{g}_{n}")
                for g in range(4) for n in range(NT)
            ]

            # HAM warmup: the PE clock-gate only opens after ~3.4us of
            # sustained matmul activity.  Burn the DMA-wait startup window on
            # dummy matmuls into psum[0] (later reset by its start=True bias
            # matmul) so the real stream runs at 2.4 GHz from the first slab.
            # memsets on the vector engine — gpsimd dispatch was measured ~6us
            # late on this kernel.
            scr = cpool.tile([128, 128 + PSH], bf16, tag="scr")
            nc.vector.memset(scr[:], 0.0)
            for _ in range(10):
                nc.tensor.matmul(
                    psum[0][:], scr[:, :128], scr[:, 128:],
                    start=True, stop=True, skip_group_check=True)

            # Bias via K=1 matmul: psum[g,n] <- ones[1,128]^T @ bias[1,512].
            # Gates f/i/o carry the 2^16 fp8 pre-scale in the host-side bias.
            bias_t = cpool.tile([1, 4 * PSH], bf16, tag="bias")
            nc.scalar.dma_start(bias_t[:], B_d.ap()[:])
            ones_t = cpool.tile([1, 128], bf16, tag="ones")
            nc.vector.memset(ones_t[:], 1.0)
            for n in range(NT):
                for g in range(4):
                    nc.tensor.matmul(
                        psum[g * NT + n][:], ones_t[:],
                        bias_t[:, g * PSH:(g + 1) * PSH],
                        start=True, stop=False)

            # SBUF-resident operands.  DMAs are issued in first-use order:
            # activations + small tensors on the scalar queue, weights on sync.
            # Globally DMA-bound: ~17 MiB over rings that only start moving
            # at ~8us and sustain ~300-400 GB/s.  Ring admission order ~= DMA
            # instruction issue order across both sequencers; every pass runs
            # off banked-ahead buffer, and late tensors (ab, w8o) are issued
            # behind the early ones so they cannot preempt them.
            a8 = apool.tile([128, QT, B], fp8, tag="a8", name="a8")
            ab = apool.tile([128, QB * B], bf16, tag="ab", name="ab")
            d8 = apool.tile([128, QB * B], fp8, tag="d8", name="d8")
            wg8 = wpool.tile([128, KG, 2, PSH], fp8, tag="wg8", name="wg8")
            w8f = wpool.tile([128, KP, 2, PSH], fp8, tag="w8f", name="w8f")
            w8i = wpool.tile([128, KP, 2, PSH], fp8, tag="w8i", name="w8i")
            w8o = wpool.tile([128, KP, 2, PSH], fp8, tag="w8o", name="w8o")
            wbg = wpool.tile([128, QB * PSH], bf16, tag="wbg", name="wbg")
            hc_t = cpool.tile([128, NT * PSH], bf16, tag="hc")

            def dma_w8(wt, g, j0, j1):
                nc.sync.dma_start(
                    wt[:, j0:j1, :, :],
                    W8_d.ap()[g][:, j0 * 2 * PSH:j1 * 2 * PSH])

            def dma_a8(q0, q1):
                nc.scalar.dma_start(a8[:, q0:q1, :], A8_d.ap()[:, q0 * B:q1 * B])

            def dma_wbg(q0, q1):
                nc.sync.dma_start(
                    wbg[:, q0 * PSH:q1 * PSH], WB_d.ap()[:, q0 * PSH:q1 * PSH])

            dma_a8(0, 2)
            dma_w8(w8f, 0, 0, 2)
            dma_a8(2, 6)
            dma_w8(w8f, 0, 2, 6)
            dma_a8(6, 16)
            dma_w8(w8f, 0, 6, 12)
            dma_a8(16, 28)
            dma_w8(w8f, 0, 12, 20)
            dma_a8(28, 40)
            for (j0, j1) in ((0, 7), (7, 14)):
                dma_w8(w8i, 1, j0, j1)
            nc.sync.dma_start(wg8[:, :, :, :], WG_d.ap()[:])
            dma_wbg(0, 8)
            dma_w8(w8i, 1, 14, 20)
            for (q0, q1) in ((8, 16), (16, 24), (24, 32)):
                dma_wbg(q0, q1)
            for (j0, j1) in ((0, 10), (10, 20)):
                dma_w8(w8o, 2, j0, j1)

            # fp8 DoubleRow stream for a sigmoid gate g (psum bank id == g).
            def fp8_gate(g, wt, n_major=False):
                loop = (
                    [(j, n) for n in range(NT) for j in range(KP)]
                    if n_major else
                    [(j, n) for j in range(KP) for n in range(NT)]
                )
                for j, n in loop:
                    nc.tensor.matmul(
                        psum[g * NT + n][:],
                        a8[:, 2 * j:2 * j + 2, n * 128:(n + 1) * 128],
                        wt[:, j, :, :],
                        start=False, stop=(j == KP - 1), perf_mode=DR)

            fp8_gate(0, w8f)                      # f
            # sigmoid(f) fires at f-stop; ab/hc sit behind it in the scalar
            # program so their ring entry (~f-end) cannot preempt w8i/wbg.
            f_a, i_a, g_a, c_t, th = [], [], [], [], []
            for n in range(NT):
                t = epool.tile([128, PSH], bf16, tag="fa", name=f"fa{n}")
                nc.scalar.activation(t[:], psum[0 * NT + n][:], AF.Sigmoid,
                                     scale=SINV)
                f_a.append(t)
            for (q0, q1) in ((0, 16), (16, 32)):
                nc.scalar.dma_start(
                    d8[:, q0 * B:q1 * B], D8_d.ap()[:, q0 * B:q1 * B])
            nc.scalar.dma_start(hc_t[:], H_d.ap()[:])

            # Reconstruct the bf16 candidate activations on the idle vector +
            # gpsimd engines instead of DMAing them: ab = a8 + d8/32, where d8
            # is the fp8-quantized residual of the fp8 activation quantization
            # (~bf16 quality at half the ring bytes).
            for ei, (q0, q1) in enumerate(((0, 8), (8, 16), (16, 24), (24, 32))):
                eng = nc.vector if ei < 2 else nc.gpsimd
                sl = slice(q0 * B, q1 * B)
                t_r = epool.tile([128, (q1 - q0) * B], bf16, tag=f"rec{ei}",
                                 name=f"rec{ei}")
                eng.tensor_scalar_mul(t_r[:], d8[:, sl], 1.0 / 32.0)
                eng.tensor_add(ab[:, sl], a8[:, q0:q1, :], t_r[:])

            fp8_gate(1, w8i)                      # i
            for n in range(NT):
                t = epool.tile([128, PSH], bf16, tag="ia", name=f"ia{n}")
                nc.scalar.activation(t[:], psum[1 * NT + n][:], AF.Sigmoid,
                                     scale=SINV)
                i_a.append(t)

            # candidate gate (psum bank 3): x/hu section (20% of the
            # variance) as fp8 DoubleRow reusing the resident a8; hidden_s
            # section in bf16 (scaled by the same 2^16 so the bank is
            # scale-uniform).
            for jj in range(KG):
                for n in range(NT):
                    j = QB // 2 + jj
                    nc.tensor.matmul(
                        psum[3 * NT + n][:],
                        a8[:, 2 * j:2 * j + 2, n * 128:(n + 1) * 128],
                        wg8[:, jj, :, :],
                        start=False, stop=False, perf_mode=DR)
            for q in range(QB):
                for n in range(NT):
                    nc.tensor.matmul(
                        psum[3 * NT + n][:],
                        ab[:, q * B + n * 128:q * B + (n + 1) * 128],
                        wbg[:, q * PSH:(q + 1) * PSH],
                        start=False, stop=(q == QB - 1))
            for n in range(NT):
                t = epool.tile([128, PSH], bf16, tag="ga", name=f"ga{n}")
                nc.scalar.activation(t[:], psum[3 * NT + n][:], AF.Tanh,
                                     scale=SINV)
                g_a.append(t)
            for n in range(NT):
                fhc = epool.tile([128, PSH], f32, tag="fhc", name=f"fhc{n}")
                nc.vector.tensor_mul(
                    fhc[:], f_a[n][:], hc_t[:, n * PSH:(n + 1) * PSH])
                ig = epool.tile([128, PSH], f32, tag="ig", name=f"ig{n}")
                nc.vector.tensor_mul(ig[:], i_a[n][:], g_a[n][:])
                ct = epool.tile([128, PSH], bf16, tag="ct", name=f"ct{n}")
                nc.vector.tensor_add(ct[:], fhc[:], ig[:])
                c_t.append(ct)
                nc.sync.dma_start(CO_d.ap()[n], ct[:])
            for n in range(NT):
                t = epool.tile([128, PSH], bf16, tag="th", name=f"th{n}")
                nc.scalar.activation(t[:], c_t[n][:], AF.Tanh)
                th.append(t)

            # o-pass runs n-major: batch-tile 0's bank closes at the halfway
            # point, so its sigmoid/mul/DMA hide under batch-tile 1's matmuls
            # and only n=1's chain remains in the tail.
            fp8_gate(2, w8o, n_major=True)        # o
            o_a = [
                epool.tile([128, PSH], bf16, tag="oa", name=f"oa{n}")
                for n in range(NT)
            ]
            os_t = [
                epool.tile([128, PSH], bf16, tag="ost", name=f"ost{n}")
                for n in range(NT)
            ]
            HP = PSH // 2
            for n in range(NT):
                for h in range(2):
                    sl = slice(h * HP, (h + 1) * HP)
                    nc.scalar.activation(
                        o_a[n][:, sl], psum[2 * NT + n][:, sl], AF.Sigmoid,
                        scale=SINV)
                    nc.vector.tensor_mul(
                        os_t[n][:, sl], o_a[n][:, sl], th[n][:, sl])
                    if n == 0 and h == 1:
                        nc.sync.dma_start(OS_d.ap()[0], os_t[0][:])
                # n=1 (the true tail) goes out in halves so the first half's
                # descriptor issue overlaps the second half's activation.
                if n == 1:
                    for h in range(2):
                        sl = slice(h * HP, (h + 1) * HP)
                        nc.sync.dma_start(
                            OS_d.ap()[1][:, sl], os_t[1][:, sl])

    nc.compile()
    return nc


def _get_nc():
    if "nc" not in _cache:
        _cache["nc"] = _build()
    return _cache["nc"]


def _prep_in_maps(x, hidden_s, hidden_u, hidden_c, Wd, Wu, Ww, Bd, Bu, Bw):
    # Activations, transposed: A_T[k, n], k = [hs (4096) | hu (512) | x (512)]
    A = np.concatenate(
        [hidden_s.reshape(B, S2), hidden_u, x], axis=1)                # [B, QC]
    A_kt = (A.T * SA).reshape(QT, 128, B)                              # [q,p,n]
    A8q = A_kt.astype(F8)
    A8 = np.ascontiguousarray(A8q.transpose(1, 0, 2)).reshape(128, QT * B)
    # fp8 residual of the hs-section activation quantization; the kernel
    # reconstructs bf16-quality candidate activations as a8 + d8/32 on-chip.
    D8 = np.ascontiguousarray(
        ((A_kt[:QB] - A8q[:QB].astype(np.float32)) * 32.0)
        .astype(F8).transpose(1, 0, 2)).reshape(128, QB * B)

    # Weights, transposed to [k, p]; contraction order [Wu | Ww | -Wd].
    # Everything is pre-scaled by SW (and activations by SA) so every PSUM
    # bank carries the same 2^16 scale; bf16 scaling by powers of 2 is exact.
    WuT = Wu.reshape(4, S2, S2).transpose(0, 2, 1)                     # [4,S2,S2]
    WwT = Ww.reshape(4, S2, U).transpose(0, 2, 1)                      # [4,U,S2]
    WdT = (-Wd.reshape(4, S2, I)).transpose(0, 2, 1)                   # [4,I,S2]
    WT = np.concatenate([WuT, WwT, WdT], axis=1) * SW                  # [4,QC,S2]
    W8_all = WT[:3].astype(F8)                                         # [3,QC,S2]
    WG_all = WT[3][QB * 128:].astype(F8)                               # [1024,S2]
    WB_all = WT[3][:QB * 128].astype(BF)                               # [4096,S2]

    bias = (Bw + Bd + Bu).reshape(4, S2).astype(np.float64) * (SA * SW)
    hc = hidden_c.reshape(NT, 128, S2)

    in_maps = []
    for c in range(NCORES):
        p0 = c * PSH
        W8_c = np.ascontiguousarray(
            W8_all[:, :, p0:p0 + PSH].reshape(3, KP, 2, 128, PSH)
            .transpose(0, 3, 1, 2, 4)).reshape(3, 128, KP * 2 * PSH)
        WG_c = np.ascontiguousarray(
            WG_all[:, p0:p0 + PSH].reshape(KG, 2, 128, PSH)
            .transpose(2, 0, 1, 3)).reshape(128, KG * 2 * PSH)
        WB_c = np.ascontiguousarray(
            WB_all[:, p0:p0 + PSH].reshape(QB, 128, PSH)
            .transpose(1, 0, 2)).reshape(128, QB * PSH)
        b_c = np.ascontiguousarray(
            bias[:, p0:p0 + PSH]).reshape(1, 4 * PSH).astype(BF)
        h_c = np.ascontiguousarray(
            hc[..., p0:p0 + PSH].transpose(1, 0, 2)).reshape(
                128, NT * PSH).astype(BF)
        in_maps.append({"A8": A8, "D8": D8, "W8": W8_c, "WG8": WG_c,
                        "WB": WB_c, "BIAS": b_c, "HC": h_c})
    return in_maps


def _run(inputs, trace=False, trace_kwargs=None):
    from concourse.bass_utils import run_bass_kernel_spmd

    nc = _get_nc()
    in_maps = _prep_in_maps(**inputs)
    res = run_bass_kernel_spmd(
        nc, in_maps, core_ids=list(range(NCORES)),
        trace=trace, **(trace_kwargs or {}))

    out_s = np.empty((B, S2), np.float32)
    c_out = np.empty((B, S2), np.float32)
    for c in range(NCORES):
        p0 = c * PSH
        out_s[:, p0:p0 + PSH] = res.results[c]["OS"].astype(
            np.float32).reshape(B, PSH)
        c_out[:, p0:p0 + PSH] = res.results[c]["CO"].astype(
            np.float32).reshape(B, PSH)
    return (out_s.reshape(B, 64, 64), c_out.reshape(B, 64, 64)), res


def kernel(**inputs):
    inputs = {k: np.asarray(v) for k, v in inputs.items()}
    (out_s, c_out), _ = _run(inputs)
    return (out_s, c_out)
